# revision 10
# baseline (speedup 1.0000x reference)
"""MoE routing kernel for Trainium2 (8 NeuronCores, expert-parallel).

Strategy:
  - Host: compute gate (sigmoid + grouped top-k routing) in numpy, gather
    tokens per expert (sparse dispatch; top-2 of 8 experts per token).
  - Device (SPMD, core e): SwiGLU MLP with expert e's weights over the
    tokens routed to e, plus a 1/8 token-shard of the shared-expert MLP.
    All matmul operands bf16 (f32 PSUM): same PE rate as f32r, half the
    DMA bytes, fully-hidden weight loads. Host pre-swizzles x/weights/out
    into [128, ...] chunk-major DRAM layouts so every DMA is a single
    contiguous-per-partition descriptor.
  - Software pipeline: the w2 phase is emitted k2-major, one h-column
    behind production, so the tensor engine never waits on the
    silu*h3 vector op (PSUM: p1 x1, p3 x1, p2 x6 banks).
  - DMA need-ordering: critical shared weights first, x prefetch behind
    them on the same queue (all DMA shares 16 engines ~360 GB/s).
  - Warmup burst of dummy matmuls trips the PE HAM clock-gate to 2.4 GHz
    during the DMA head, so real matmuls start at full clock.
  - Host: weighted scatter-add of expert outputs + shared output.
"""

import numpy as np
import ml_dtypes
from contextlib import ExitStack

DIM = 768
INTER = 512
E = 8
G = 4
TOPK = 2
N_CORES = 8
P = 128
NCHUNK = 512  # tokens per PSUM tile (fp32 bank limit)
KD = DIM // P  # 6 k-tiles over model dim
KI = INTER // P  # 4 k-tiles over inter dim
WARMUP_MM = 100

BF16 = ml_dtypes.bfloat16


# ---------------------------------------------------------------- host gate
def _host_gate(x2, gate_weight, gate_bias):
    """Reproduces reference._gate in numpy f32. Returns (w [T,2], idx [T,2])."""
    T = x2.shape[0]
    logits = x2 @ gate_weight.T
    scores = 1.0 / (1.0 + np.exp(-logits, dtype=np.float32))
    s = scores + gate_bias
    sv = s.reshape(T, G, E // G)
    group_scores = sv.sum(-1)  # top-2 of 2 per group == sum
    gidx = np.argsort(-group_scores, axis=1, kind="stable")[:, :2]
    gmask = np.zeros((T, G), bool)
    gmask[np.arange(T)[:, None], gidx] = True
    masked = np.where(gmask[:, :, None], sv, -np.inf).reshape(T, E)
    idx = np.argsort(-masked, axis=1, kind="stable")[:, :TOPK]
    w = np.take_along_axis(scores, idx, axis=1)
    w = w / (w.sum(-1, keepdims=True) + 1e-6)
    return w.astype(np.float32), idx.astype(np.int32)


# --------------------------------------------------------- host swizzle utils
def _chunks_of(n):
    """Chunk sizes covering n tokens: full NCHUNKs then one remainder."""
    out = []
    r = n
    while r > 0:
        c = min(NCHUNK, r)
        out.append(c)
        r -= c
    return out


def _swizzle_x(x2T_pad):
    """[DIM, n] -> [P, sum(KD*c)] chunk-major bf16 ([KD, c] per chunk)."""
    n = x2T_pad.shape[1]
    pieces = []
    off = 0
    for c in _chunks_of(n):
        blk = x2T_pad[:, off : off + c]  # [DIM, c]
        blk = blk.reshape(KD, P, c).transpose(1, 0, 2).reshape(P, KD * c)
        pieces.append(blk)
        off += c
    return np.ascontiguousarray(np.concatenate(pieces, axis=1).astype(BF16))


def _swizzle_w13(wT):
    """[DIM, INTER] (w.T) -> [P, KI, KD, P] m-major bf16, flattened."""
    a = wT.reshape(KD, P, KI, P).transpose(1, 2, 0, 3)  # [P, KI, KD, P]
    return np.ascontiguousarray(a.reshape(P, KI * KD * P).astype(BF16))


def _swizzle_w2(w2T):
    """[INTER, DIM] (w2.T) -> [P, KD, KI, P] m-major bf16, flattened."""
    a = w2T.reshape(KI, P, KD, P).transpose(1, 2, 0, 3)  # [P, KD, KI, P]
    return np.ascontiguousarray(a.reshape(P, KD * KI * P).astype(BF16))


def _unswizzle_out(o, n):
    """[P, sum(KD*c)] bf16 chunk-major -> [n, DIM] f32."""
    cols = []
    off = 0
    o = o.astype(np.float32)
    for c in _chunks_of(n):
        blk = o[:, off : off + KD * c].reshape(P, KD, c)
        blk = blk.transpose(1, 0, 2).reshape(DIM, c)
        cols.append(blk)
        off += KD * c
    return np.concatenate(cols, axis=1).T  # [n, DIM]


# ---------------------------------------------------------- device kernel IR
def _build_nc(cap, nsh):
    import concourse.bass as bass
    import concourse.tile as tile
    from concourse import bacc, mybir

    f32 = mybir.dt.float32
    bf = mybir.dt.bfloat16

    nc = bacc.Bacc(
        "TRN2",
        target_bir_lowering=False,
        debug=False,
        enable_asserts=False,
        num_devices=N_CORES,
    )

    def xfree(n):
        return sum(KD * c for c in _chunks_of(n))

    xg = nc.dram_tensor("xg", [P, xfree(cap)], bf, kind="ExternalInput").ap()
    xs = nc.dram_tensor("xs", [P, xfree(nsh)], bf, kind="ExternalInput").ap()
    w1t = nc.dram_tensor("w1t", [P, KI * KD * P], bf, kind="ExternalInput").ap()
    w3t = nc.dram_tensor("w3t", [P, KI * KD * P], bf, kind="ExternalInput").ap()
    w2t = nc.dram_tensor("w2t", [P, KD * KI * P], bf, kind="ExternalInput").ap()
    sw1t = nc.dram_tensor("sw1t", [P, KI * KD * P], bf, kind="ExternalInput").ap()
    sw3t = nc.dram_tensor("sw3t", [P, KI * KD * P], bf, kind="ExternalInput").ap()
    sw2t = nc.dram_tensor("sw2t", [P, KD * KI * P], bf, kind="ExternalInput").ap()
    oe = nc.dram_tensor("oe", [P, xfree(cap)], bf, kind="ExternalOutput").ap()
    oz = nc.dram_tensor("oz", [P, xfree(nsh)], bf, kind="ExternalOutput").ap()

    with tile.TileContext(nc) as tc, ExitStack() as ctx:
        wpool = ctx.enter_context(tc.tile_pool(name="wpool", bufs=1))
        xpool = ctx.enter_context(tc.tile_pool(name="xpool", bufs=4))
        hpool = ctx.enter_context(tc.tile_pool(name="hpool", bufs=3))
        sgpool = ctx.enter_context(tc.tile_pool(name="sgpool", bufs=3))
        opool = ctx.enter_context(tc.tile_pool(name="opool", bufs=4))
        # PSUM banks: p1 x1, p3 x1, p2 x6 (8 total)
        p1pool = ctx.enter_context(tc.tile_pool(name="p1pool", bufs=1, space="PSUM"))
        p3pool = ctx.enter_context(tc.tile_pool(name="p3pool", bufs=1, space="PSUM"))
        p2pool = ctx.enter_context(tc.tile_pool(name="p2pool", bufs=6, space="PSUM"))

        # ---- HAM warmup: dummy matmuls to trip the PE clock to 2.4 GHz ----
        wz = wpool.tile([P, 64], bf, tag="wz", name="wz")
        nc.vector.memset(wz, 0.0)
        pwarm = p2pool.tile([P, NCHUNK], f32, tag="p2", name="pwarm")
        for i in range(WARMUP_MM):
            nc.tensor.matmul(
                pwarm[:64, :64],
                wz[:, :64],
                wz[:, :64],
                start=(i == 0),
                stop=(i == WARMUP_MM - 1),
            )

        # ---- weight tiles (persistent, m-major) ----
        sw1s = wpool.tile([P, KI, KD, P], bf, tag="sw1s", name="sw1s")
        sw3s = wpool.tile([P, KI, KD, P], bf, tag="sw3s", name="sw3s")
        sw2s = wpool.tile([P, KD, KI, P], bf, tag="sw2s", name="sw2s")
        w1s = wpool.tile([P, KI, KD, P], bf, tag="w1s", name="w1s")
        w3s = wpool.tile([P, KI, KD, P], bf, tag="w3s", name="w3s")
        w2s = wpool.tile([P, KD, KI, P], bf, tag="w2s", name="w2s")

        # ---- unified chunk schedule over both phases ----
        sched = []  # (xT, outT, a1, a3, a2, xoff, n)
        xoff = 0
        for n in _chunks_of(nsh):
            sched.append((xs, oz, sw1s, sw3s, sw2s, xoff, n))
            xoff += KD * n
        xoff = 0
        for n in _chunks_of(cap):
            sched.append((xg, oe, w1s, w3s, w2s, xoff, n))
            xoff += KD * n
        nchunks = len(sched)

        # ---- DMA issue, need-ordered ----
        # x chunk 0 on the sync queue (2 halves); everything else on the
        # gpsimd queue in consumption order: shared-weight m-blocks first,
        # then sw2, x1, x2, routed weights, then the x tail.  All queues
        # feed the same 16 DMA engines; per-queue FIFO order is what
        # controls which bytes land first.
        xtiles = [None] * nchunks

        def issue_x(c, queue, split=False):
            xT, _, _, _, _, xoff, n = sched[c]
            xt = xpool.tile([P, KD, NCHUNK], bf, tag="xt", name="xt")
            src = xT[:, xoff : xoff + KD * n].rearrange("p (kt n) -> p kt n", kt=KD)
            if split:
                hh = KD // 2
                queue.dma_start(out=xt[:, :hh, :n], in_=src[:, :hh, :])
                queue.dma_start(out=xt[:, hh:, :n], in_=src[:, hh:, :])
            else:
                queue.dma_start(out=xt[:, :, :n], in_=src)
            xtiles[c] = xt

        issue_x(0, nc.sync, split=True)

        sw1r = sw1t.rearrange("p (mb k mi) -> p mb k mi", mb=KI, k=KD)
        sw3r = sw3t.rearrange("p (mb k mi) -> p mb k mi", mb=KI, k=KD)
        for mb in range(KI):
            nc.gpsimd.dma_start(out=sw1s[:, mb], in_=sw1r[:, mb])
            nc.gpsimd.dma_start(out=sw3s[:, mb], in_=sw3r[:, mb])
        nc.gpsimd.dma_start(
            out=sw2s[:, :, :, :],
            in_=sw2t.rearrange("p (mb k mi) -> p mb k mi", mb=KD, k=KI),
        )
        issue_x(1, nc.gpsimd)
        issue_x(2, nc.gpsimd)
        for t, src, mbn, kn in (
            (w1s, w1t, KI, KD),
            (w3s, w3t, KI, KD),
            (w2s, w2t, KD, KI),
        ):
            nc.gpsimd.dma_start(
                out=t[:, :, :, :],
                in_=src.rearrange("p (mb k mi) -> p mb k mi", mb=mbn, k=kn),
            )

        Silu = mybir.ActivationFunctionType.Silu
        Copy = mybir.ActivationFunctionType.Copy

        htiles = [None] * nchunks
        p2ts = [None] * nchunks

        def emit_batch(c, k2):
            """k2-th accumulation slice into all KD p2 banks of chunk c."""
            _, outT, _, _, a2, xoff, n = sched[c]
            if p2ts[c] is None:
                p2ts[c] = [
                    p2pool.tile([P, NCHUNK], f32, tag="p2", name="p2")
                    for _ in range(KD)
                ]
            h = htiles[c]
            last = k2 == KI - 1
            ot = None
            if last:
                ot = opool.tile([P, KD, NCHUNK], bf, tag="ot", name="ot")
            for m2 in range(KD):
                nc.tensor.matmul(
                    p2ts[c][m2][:, :n],
                    a2[:, m2, k2, :],
                    h[:, k2, :n],
                    start=(k2 == 0),
                    stop=last,
                )
                if last:
                    # copy emitted right behind its own stop-matmul so its
                    # semaphore target is per-bank precise
                    if m2 % 2 == 0:
                        nc.scalar.activation(
                            ot[:, m2, :n], p2ts[c][m2][:, :n], Copy
                        )
                    else:
                        nc.vector.tensor_copy(ot[:, m2, :n], p2ts[c][m2][:, :n])
            if last:
                dst = outT[:, xoff : xoff + KD * n].rearrange(
                    "p (kt n) -> p kt n", kt=KD
                )
                nc.sync.dma_start(out=dst, in_=ot[:, :, :n])
                htiles[c] = None
                p2ts[c] = None

        prev = None  # (c, m) one h-step behind
        for c in range(nchunks):
            _, _, a1, a3, _, _, n = sched[c]
            if c + 3 < nchunks:
                issue_x(c + 3, nc.sync)
            xt = xtiles[c]
            htiles[c] = hpool.tile([P, KI, NCHUNK], bf, tag="h", name="h")
            for m in range(KI):
                p1 = p1pool.tile([P, NCHUNK], f32, tag="p1", name="p1")
                for k in range(KD):
                    nc.tensor.matmul(
                        p1[:, :n],
                        a1[:, m, k, :],
                        xt[:, k, :n],
                        start=(k == 0),
                        stop=(k == KD - 1),
                    )
                sg = sgpool.tile([P, NCHUNK], bf, tag="sg", name="sg")
                nc.scalar.activation(sg[:, :n], p1[:, :n], Silu)
                p3 = p3pool.tile([P, NCHUNK], f32, tag="p3", name="p3")
                for k in range(KD):
                    nc.tensor.matmul(
                        p3[:, :n],
                        a3[:, m, k, :],
                        xt[:, k, :n],
                        start=(k == 0),
                        stop=(k == KD - 1),
                    )
                nc.vector.tensor_mul(
                    htiles[c][:, m, :n], sg[:, :n], p3[:, :n]
                )
                if prev is not None:
                    emit_batch(*prev)
                prev = (c, m)
            xtiles[c] = None
        emit_batch(*prev)

    nc.compile()
    return nc


# ------------------------------------------------------------------- driver
def kernel(x, gate_weight, gate_bias, w1, w2, w3, sw1, sw2, sw3):
    from concourse.bass_utils import run_bass_kernel_spmd

    B, S, D = x.shape
    x2 = np.ascontiguousarray(x.reshape(-1, D))
    T = x2.shape[0]
    nsh = T // N_CORES

    w, idx = _host_gate(x2, gate_weight, gate_bias)

    rows_per_e = [np.nonzero((idx == e).any(axis=1))[0] for e in range(E)]
    cap = max(len(r) for r in rows_per_e)
    cap = ((cap + P - 1) // P) * P

    nc = _build_nc(cap, nsh)

    x2T = np.ascontiguousarray(x2.T)  # [D, T]
    in_maps = []
    for e in range(E):
        rows = rows_per_e[e]
        xgT = np.zeros((DIM, cap), np.float32)
        xgT[:, : len(rows)] = x2T[:, rows]
        in_maps.append(
            {
                "xg": _swizzle_x(xgT),
                "xs": _swizzle_x(x2T[:, e * nsh : (e + 1) * nsh]),
                "w1t": _swizzle_w13(w1[e].T),
                "w3t": _swizzle_w13(w3[e].T),
                "w2t": _swizzle_w2(w2[e].T),
                "sw1t": _swizzle_w13(sw1.T),
                "sw3t": _swizzle_w13(sw3.T),
                "sw2t": _swizzle_w2(sw2.T),
            }
        )

    r = run_bass_kernel_spmd(nc, in_maps, list(range(N_CORES)))
    globals()["LAST_RESULTS"] = r
    res = r.results

    y = np.zeros((T, D), np.float32)
    for e in range(E):
        rows = rows_per_e[e]
        cnt = len(rows)
        Oe = _unswizzle_out(res[e]["oe"], cap)[:cnt]  # [cnt, D]
        we = np.where(idx[rows, 0] == e, w[rows, 0], w[rows, 1]).astype(np.float32)
        y[rows] += we[:, None] * Oe
    z = np.concatenate(
        [_unswizzle_out(res[c]["oz"], nsh) for c in range(N_CORES)], axis=0
    )
    return (y + z).reshape(B, S, D)


# revision 14
# speedup vs baseline: 1.0037x; 1.0037x over previous
"""MoE routing kernel for Trainium2 (8 NeuronCores, expert-parallel).

Strategy:
  - Host: compute gate (sigmoid + grouped top-k routing) in numpy, gather
    tokens per expert (sparse dispatch; top-2 of 8 experts per token).
  - Device (SPMD, core e): SwiGLU MLP with expert e's weights over the
    tokens routed to e, plus a 1/8 token-shard of the shared-expert MLP.
    All matmul operands bf16 (f32 PSUM): same PE rate as f32r, half the
    DMA bytes, fully-hidden weight loads. Host pre-swizzles x/weights/out
    into [128, ...] chunk-major DRAM layouts so every DMA is a single
    contiguous-per-partition descriptor.
  - Software pipeline: the w2 phase is emitted k2-major, one h-column
    behind production, so the tensor engine never waits on the
    silu*h3 vector op (PSUM: p1 x1, p3 x1, p2 x6 banks).
  - DMA need-ordering: critical shared weights first, x prefetch behind
    them on the same queue (all DMA shares 16 engines ~360 GB/s).
  - Warmup burst of dummy matmuls trips the PE HAM clock-gate to 2.4 GHz
    during the DMA head, so real matmuls start at full clock.
  - Host: weighted scatter-add of expert outputs + shared output.
"""

import numpy as np
import ml_dtypes
from contextlib import ExitStack

DIM = 768
INTER = 512
E = 8
G = 4
TOPK = 2
N_CORES = 8
P = 128
NCHUNK = 512  # tokens per PSUM tile (fp32 bank limit)
KD = DIM // P  # 6 k-tiles over model dim
KI = INTER // P  # 4 k-tiles over inter dim
WARMUP_MM = 100

BF16 = ml_dtypes.bfloat16


# ---------------------------------------------------------------- host gate
def _host_gate(x2, gate_weight, gate_bias):
    """Reproduces reference._gate in numpy f32. Returns (w [T,2], idx [T,2])."""
    T = x2.shape[0]
    logits = x2 @ gate_weight.T
    scores = 1.0 / (1.0 + np.exp(-logits, dtype=np.float32))
    s = scores + gate_bias
    sv = s.reshape(T, G, E // G)
    group_scores = sv.sum(-1)  # top-2 of 2 per group == sum
    gidx = np.argsort(-group_scores, axis=1, kind="stable")[:, :2]
    gmask = np.zeros((T, G), bool)
    gmask[np.arange(T)[:, None], gidx] = True
    masked = np.where(gmask[:, :, None], sv, -np.inf).reshape(T, E)
    idx = np.argsort(-masked, axis=1, kind="stable")[:, :TOPK]
    w = np.take_along_axis(scores, idx, axis=1)
    w = w / (w.sum(-1, keepdims=True) + 1e-6)
    return w.astype(np.float32), idx.astype(np.int32)


# --------------------------------------------------------- host swizzle utils
def _chunks_of(n):
    """Chunk sizes covering n tokens: full NCHUNKs then one remainder."""
    out = []
    r = n
    while r > 0:
        c = min(NCHUNK, r)
        out.append(c)
        r -= c
    return out


def _swizzle_x(x2T_pad):
    """[DIM, n] -> [P, sum(KD*c)] chunk-major bf16 ([KD, c] per chunk)."""
    n = x2T_pad.shape[1]
    pieces = []
    off = 0
    for c in _chunks_of(n):
        blk = x2T_pad[:, off : off + c]  # [DIM, c]
        blk = blk.reshape(KD, P, c).transpose(1, 0, 2).reshape(P, KD * c)
        pieces.append(blk)
        off += c
    return np.ascontiguousarray(np.concatenate(pieces, axis=1).astype(BF16))


def _swizzle_w13(wT):
    """[DIM, INTER] (w.T) -> [P, KI, KD, P] m-major bf16, flattened."""
    a = wT.reshape(KD, P, KI, P).transpose(1, 2, 0, 3)  # [P, KI, KD, P]
    return np.ascontiguousarray(a.reshape(P, KI * KD * P).astype(BF16))


def _swizzle_w2(w2T):
    """[INTER, DIM] (w2.T) -> [P, KD, KI, P] m-major bf16, flattened."""
    a = w2T.reshape(KI, P, KD, P).transpose(1, 2, 0, 3)  # [P, KD, KI, P]
    return np.ascontiguousarray(a.reshape(P, KD * KI * P).astype(BF16))


def _unswizzle_out(o, n):
    """[P, sum(KD*c)] bf16 chunk-major -> [n, DIM] f32."""
    cols = []
    off = 0
    o = o.astype(np.float32)
    for c in _chunks_of(n):
        blk = o[:, off : off + KD * c].reshape(P, KD, c)
        blk = blk.transpose(1, 0, 2).reshape(DIM, c)
        cols.append(blk)
        off += KD * c
    return np.concatenate(cols, axis=1).T  # [n, DIM]


# ---------------------------------------------------------- device kernel IR
def _build_nc(cap, nsh):
    import concourse.bass as bass
    import concourse.tile as tile
    from concourse import bacc, mybir

    f32 = mybir.dt.float32
    bf = mybir.dt.bfloat16

    nc = bacc.Bacc(
        "TRN2",
        target_bir_lowering=False,
        debug=False,
        enable_asserts=False,
        num_devices=N_CORES,
    )

    def xfree(n):
        return sum(KD * c for c in _chunks_of(n))

    xg = nc.dram_tensor("xg", [P, xfree(cap)], bf, kind="ExternalInput").ap()
    xs = nc.dram_tensor("xs", [P, xfree(nsh)], bf, kind="ExternalInput").ap()
    w1t = nc.dram_tensor("w1t", [P, KI * KD * P], bf, kind="ExternalInput").ap()
    w3t = nc.dram_tensor("w3t", [P, KI * KD * P], bf, kind="ExternalInput").ap()
    w2t = nc.dram_tensor("w2t", [P, KD * KI * P], bf, kind="ExternalInput").ap()
    sw1t = nc.dram_tensor("sw1t", [P, KI * KD * P], bf, kind="ExternalInput").ap()
    sw3t = nc.dram_tensor("sw3t", [P, KI * KD * P], bf, kind="ExternalInput").ap()
    sw2t = nc.dram_tensor("sw2t", [P, KD * KI * P], bf, kind="ExternalInput").ap()
    oe = nc.dram_tensor("oe", [P, xfree(cap)], bf, kind="ExternalOutput").ap()
    oz = nc.dram_tensor("oz", [P, xfree(nsh)], bf, kind="ExternalOutput").ap()

    with tile.TileContext(nc) as tc, ExitStack() as ctx:
        wpool = ctx.enter_context(tc.tile_pool(name="wpool", bufs=1))
        xpool = ctx.enter_context(tc.tile_pool(name="xpool", bufs=4))
        hpool = ctx.enter_context(tc.tile_pool(name="hpool", bufs=3))
        sgpool = ctx.enter_context(tc.tile_pool(name="sgpool", bufs=3))
        opool = ctx.enter_context(tc.tile_pool(name="opool", bufs=4))
        # PSUM banks: p1 x1, p3 x1, p2 x6 (8 total)
        p1pool = ctx.enter_context(tc.tile_pool(name="p1pool", bufs=1, space="PSUM"))
        p3pool = ctx.enter_context(tc.tile_pool(name="p3pool", bufs=1, space="PSUM"))
        p2pool = ctx.enter_context(tc.tile_pool(name="p2pool", bufs=6, space="PSUM"))

        # ---- HAM warmup: dummy matmuls to trip the PE clock to 2.4 GHz ----
        wz = wpool.tile([P, 64], bf, tag="wz", name="wz")
        nc.vector.memset(wz, 0.0)
        pwarm = p2pool.tile([P, NCHUNK], f32, tag="p2", name="pwarm")
        for i in range(WARMUP_MM):
            nc.tensor.matmul(
                pwarm[:64, :64],
                wz[:, :64],
                wz[:, :64],
                start=(i == 0),
                stop=(i == WARMUP_MM - 1),
            )

        # ---- weight tiles (persistent, m-major) ----
        sw1s = wpool.tile([P, KI, KD, P], bf, tag="sw1s", name="sw1s")
        sw3s = wpool.tile([P, KI, KD, P], bf, tag="sw3s", name="sw3s")
        sw2s = wpool.tile([P, KD, KI, P], bf, tag="sw2s", name="sw2s")
        w1s = wpool.tile([P, KI, KD, P], bf, tag="w1s", name="w1s")
        w3s = wpool.tile([P, KI, KD, P], bf, tag="w3s", name="w3s")
        w2s = wpool.tile([P, KD, KI, P], bf, tag="w2s", name="w2s")

        # ---- unified chunk schedule over both phases ----
        sched = []  # (xT, outT, a1, a3, a2, xoff, n)
        xoff = 0
        for n in _chunks_of(nsh):
            sched.append((xs, oz, sw1s, sw3s, sw2s, xoff, n))
            xoff += KD * n
        xoff = 0
        for n in _chunks_of(cap):
            sched.append((xg, oe, w1s, w3s, w2s, xoff, n))
            xoff += KD * n
        nchunks = len(sched)

        # ---- DMA issue, need-ordered ----
        # x chunk 0 on the sync queue (2 halves); everything else on the
        # gpsimd queue in consumption order: shared-weight m-blocks first,
        # then sw2, x1, x2, routed weights, then the x tail.  All queues
        # feed the same 16 DMA engines; per-queue FIFO order is what
        # controls which bytes land first.
        xtiles = [None] * nchunks

        def issue_x(c, queue, split=False):
            xT, _, _, _, _, xoff, n = sched[c]
            xt = xpool.tile([P, KD, NCHUNK], bf, tag="xt", name="xt")
            src = xT[:, xoff : xoff + KD * n].rearrange("p (kt n) -> p kt n", kt=KD)
            if split:
                hh = KD // 2
                queue.dma_start(out=xt[:, :hh, :n], in_=src[:, :hh, :])
                queue.dma_start(out=xt[:, hh:, :n], in_=src[:, hh:, :])
            else:
                queue.dma_start(out=xt[:, :, :n], in_=src)
            xtiles[c] = xt

        issue_x(0, nc.sync, split=True)

        sw1r = sw1t.rearrange("p (mb k mi) -> p mb k mi", mb=KI, k=KD)
        sw3r = sw3t.rearrange("p (mb k mi) -> p mb k mi", mb=KI, k=KD)
        for mb in range(KI):
            nc.gpsimd.dma_start(out=sw1s[:, mb], in_=sw1r[:, mb])
            nc.gpsimd.dma_start(out=sw3s[:, mb], in_=sw3r[:, mb])
        nc.gpsimd.dma_start(
            out=sw2s[:, :, :, :],
            in_=sw2t.rearrange("p (mb k mi) -> p mb k mi", mb=KD, k=KI),
        )
        issue_x(1, nc.gpsimd)
        issue_x(2, nc.gpsimd)
        for t, src, mbn, kn in (
            (w1s, w1t, KI, KD),
            (w3s, w3t, KI, KD),
            (w2s, w2t, KD, KI),
        ):
            nc.gpsimd.dma_start(
                out=t[:, :, :, :],
                in_=src.rearrange("p (mb k mi) -> p mb k mi", mb=mbn, k=kn),
            )

        Silu = mybir.ActivationFunctionType.Silu
        Copy = mybir.ActivationFunctionType.Copy

        htiles = [None] * nchunks
        p2ts = [None] * nchunks

        def emit_batch(c, k2):
            """k2-th accumulation slice into all KD p2 banks of chunk c."""
            _, outT, _, _, a2, xoff, n = sched[c]
            if p2ts[c] is None:
                p2ts[c] = [
                    p2pool.tile([P, NCHUNK], f32, tag="p2", name="p2")
                    for _ in range(KD)
                ]
            h = htiles[c]
            last = k2 == KI - 1
            ot = None
            if last:
                ot = opool.tile([P, KD, NCHUNK], bf, tag="ot", name="ot")
            for m2 in range(KD):
                nc.tensor.matmul(
                    p2ts[c][m2][:, :n],
                    a2[:, m2, k2, :],
                    h[:, k2, :n],
                    start=(k2 == 0),
                    stop=last,
                )
                if last:
                    # copy emitted right behind its own stop-matmul so its
                    # semaphore target is per-bank precise
                    if m2 % 2 == 0:
                        nc.scalar.activation(
                            ot[:, m2, :n], p2ts[c][m2][:, :n], Copy
                        )
                    else:
                        nc.vector.tensor_copy(ot[:, m2, :n], p2ts[c][m2][:, :n])
            if last:
                dst = outT[:, xoff : xoff + KD * n].rearrange(
                    "p (kt n) -> p kt n", kt=KD
                )
                nc.sync.dma_start(out=dst, in_=ot[:, :, :n])
                htiles[c] = None
                p2ts[c] = None
                # prefetch issue sits behind this out-DMA on the sync queue,
                # so its transfer can't steal head bandwidth
                if c + 3 < nchunks:
                    issue_x(c + 3, nc.sync)

        prev = None  # (c, m) one h-step behind
        for c in range(nchunks):
            _, _, a1, a3, _, _, n = sched[c]
            xt = xtiles[c]
            htiles[c] = hpool.tile([P, KI, NCHUNK], bf, tag="h", name="h")
            for m in range(KI):
                p1 = p1pool.tile([P, NCHUNK], f32, tag="p1", name="p1")
                for k in range(KD):
                    nc.tensor.matmul(
                        p1[:, :n],
                        a1[:, m, k, :],
                        xt[:, k, :n],
                        start=(k == 0),
                        stop=(k == KD - 1),
                    )
                sg = sgpool.tile([P, NCHUNK], bf, tag="sg", name="sg")
                nc.scalar.activation(sg[:, :n], p1[:, :n], Silu)
                p3 = p3pool.tile([P, NCHUNK], f32, tag="p3", name="p3")
                for k in range(KD):
                    nc.tensor.matmul(
                        p3[:, :n],
                        a3[:, m, k, :],
                        xt[:, k, :n],
                        start=(k == 0),
                        stop=(k == KD - 1),
                    )
                nc.vector.tensor_mul(
                    htiles[c][:, m, :n], sg[:, :n], p3[:, :n]
                )
                if prev is not None:
                    emit_batch(*prev)
                prev = (c, m)
            xtiles[c] = None
        emit_batch(*prev)

    nc.compile()
    return nc


# ------------------------------------------------------------------- driver
def kernel(x, gate_weight, gate_bias, w1, w2, w3, sw1, sw2, sw3):
    from concourse.bass_utils import run_bass_kernel_spmd

    B, S, D = x.shape
    x2 = np.ascontiguousarray(x.reshape(-1, D))
    T = x2.shape[0]
    nsh = T // N_CORES

    w, idx = _host_gate(x2, gate_weight, gate_bias)

    rows_per_e = [np.nonzero((idx == e).any(axis=1))[0] for e in range(E)]
    cap = max(len(r) for r in rows_per_e)
    cap = ((cap + P - 1) // P) * P

    nc = _build_nc(cap, nsh)

    x2T = np.ascontiguousarray(x2.T)  # [D, T]
    in_maps = []
    for e in range(E):
        rows = rows_per_e[e]
        xgT = np.zeros((DIM, cap), np.float32)
        xgT[:, : len(rows)] = x2T[:, rows]
        in_maps.append(
            {
                "xg": _swizzle_x(xgT),
                "xs": _swizzle_x(x2T[:, e * nsh : (e + 1) * nsh]),
                "w1t": _swizzle_w13(w1[e].T),
                "w3t": _swizzle_w13(w3[e].T),
                "w2t": _swizzle_w2(w2[e].T),
                "sw1t": _swizzle_w13(sw1.T),
                "sw3t": _swizzle_w13(sw3.T),
                "sw2t": _swizzle_w2(sw2.T),
            }
        )

    r = run_bass_kernel_spmd(nc, in_maps, list(range(N_CORES)))
    globals()["LAST_RESULTS"] = r
    res = r.results

    y = np.zeros((T, D), np.float32)
    for e in range(E):
        rows = rows_per_e[e]
        cnt = len(rows)
        Oe = _unswizzle_out(res[e]["oe"], cap)[:cnt]  # [cnt, D]
        we = np.where(idx[rows, 0] == e, w[rows, 0], w[rows, 1]).astype(np.float32)
        y[rows] += we[:, None] * Oe
    z = np.concatenate(
        [_unswizzle_out(res[c]["oz"], nsh) for c in range(N_CORES)], axis=0
    )
    return (y + z).reshape(B, S, D)


# revision 21
# speedup vs baseline: 1.0237x; 1.0199x over previous
"""MoE routing kernel for Trainium2 (8 NeuronCores, expert-parallel).

Strategy:
  - Host: compute gate (sigmoid + grouped top-k routing) in numpy, gather
    tokens per expert (sparse dispatch; top-2 of 8 experts per token).
  - Device (SPMD, core e): SwiGLU MLP with expert e's weights over the
    tokens routed to e, plus a 1/8 token-shard of the shared-expert MLP.
    All matmul operands bf16 (f32 PSUM): same PE rate as f32r, half the
    DMA bytes, fully-hidden weight loads. Host pre-swizzles x/weights/out
    into [128, ...] chunk-major DRAM layouts so every DMA is a single
    contiguous-per-partition descriptor.
  - Software pipeline: the w2 phase is emitted k2-major, one h-column
    behind production, so the tensor engine never waits on the
    silu*h3 vector op (PSUM: p1 x1, p3 x1, p2 x6 banks).
  - DMA need-ordering: critical shared weights first, x prefetch behind
    them on the same queue (all DMA shares 16 engines ~360 GB/s).
  - Warmup burst of dummy matmuls trips the PE HAM clock-gate to 2.4 GHz
    during the DMA head, so real matmuls start at full clock.
  - Host: weighted scatter-add of expert outputs + shared output.
"""

import numpy as np
import ml_dtypes
from contextlib import ExitStack

DIM = 768
INTER = 512
E = 8
G = 4
TOPK = 2
N_CORES = 8
P = 128
NCHUNK = 512  # tokens per PSUM tile (fp32 bank limit)
KD = DIM // P  # 6 k-tiles over model dim
KI = INTER // P  # 4 k-tiles over inter dim
WARMUP_MM = 90

BF16 = ml_dtypes.bfloat16


# ---------------------------------------------------------------- host gate
def _host_gate(x2, gate_weight, gate_bias):
    """Reproduces reference._gate in numpy f32. Returns (w [T,2], idx [T,2])."""
    T = x2.shape[0]
    logits = x2 @ gate_weight.T
    scores = 1.0 / (1.0 + np.exp(-logits, dtype=np.float32))
    s = scores + gate_bias
    sv = s.reshape(T, G, E // G)
    group_scores = sv.sum(-1)  # top-2 of 2 per group == sum
    gidx = np.argsort(-group_scores, axis=1, kind="stable")[:, :2]
    gmask = np.zeros((T, G), bool)
    gmask[np.arange(T)[:, None], gidx] = True
    masked = np.where(gmask[:, :, None], sv, -np.inf).reshape(T, E)
    idx = np.argsort(-masked, axis=1, kind="stable")[:, :TOPK]
    w = np.take_along_axis(scores, idx, axis=1)
    w = w / (w.sum(-1, keepdims=True) + 1e-6)
    return w.astype(np.float32), idx.astype(np.int32)


# --------------------------------------------------------- host swizzle utils
def _chunks_of(n):
    """Chunk sizes covering n tokens: full NCHUNKs then one remainder."""
    out = []
    r = n
    while r > 0:
        c = min(NCHUNK, r)
        out.append(c)
        r -= c
    return out


def _swizzle_x(x2T_pad):
    """[DIM, n] -> [P, sum(KD*c)] chunk-major bf16 ([KD, c] per chunk)."""
    n = x2T_pad.shape[1]
    pieces = []
    off = 0
    for c in _chunks_of(n):
        blk = x2T_pad[:, off : off + c]  # [DIM, c]
        blk = blk.reshape(KD, P, c).transpose(1, 0, 2).reshape(P, KD * c)
        pieces.append(blk)
        off += c
    return np.ascontiguousarray(np.concatenate(pieces, axis=1).astype(BF16))


def _swizzle_w13(wT):
    """[DIM, INTER] (w.T) -> [P, KI, KD, P] m-major bf16, flattened."""
    a = wT.reshape(KD, P, KI, P).transpose(1, 2, 0, 3)  # [P, KI, KD, P]
    return np.ascontiguousarray(a.reshape(P, KI * KD * P).astype(BF16))


def _swizzle_w2(w2T):
    """[INTER, DIM] (w2.T) -> [P, KI, KD, P] k2-major bf16, flattened."""
    a = w2T.reshape(KI, P, KD, P).transpose(1, 0, 2, 3)  # [P, KI, KD, P]
    return np.ascontiguousarray(a.reshape(P, KI * KD * P).astype(BF16))


def _unswizzle_out(o, n):
    """[P, sum(KD*c)] bf16 chunk-major -> [n, DIM] f32."""
    cols = []
    off = 0
    o = o.astype(np.float32)
    for c in _chunks_of(n):
        blk = o[:, off : off + KD * c].reshape(P, KD, c)
        blk = blk.transpose(1, 0, 2).reshape(DIM, c)
        cols.append(blk)
        off += KD * c
    return np.concatenate(cols, axis=1).T  # [n, DIM]


# ---------------------------------------------------------- device kernel IR
def _build_nc(cap, nsh):
    import concourse.bass as bass
    import concourse.tile as tile
    from concourse import bacc, mybir

    f32 = mybir.dt.float32
    bf = mybir.dt.bfloat16

    nc = bacc.Bacc(
        "TRN2",
        target_bir_lowering=False,
        debug=False,
        enable_asserts=False,
        num_devices=N_CORES,
    )

    def xfree(n):
        return sum(KD * c for c in _chunks_of(n))

    xg = nc.dram_tensor("xg", [P, xfree(cap)], bf, kind="ExternalInput").ap()
    xs = nc.dram_tensor("xs", [P, xfree(nsh)], bf, kind="ExternalInput").ap()
    w1t = nc.dram_tensor("w1t", [P, KI * KD * P], bf, kind="ExternalInput").ap()
    w3t = nc.dram_tensor("w3t", [P, KI * KD * P], bf, kind="ExternalInput").ap()
    w2t = nc.dram_tensor("w2t", [P, KI * KD * P], bf, kind="ExternalInput").ap()
    sw1t = nc.dram_tensor("sw1t", [P, KI * KD * P], bf, kind="ExternalInput").ap()
    sw3t = nc.dram_tensor("sw3t", [P, KI * KD * P], bf, kind="ExternalInput").ap()
    sw2t = nc.dram_tensor("sw2t", [P, KI * KD * P], bf, kind="ExternalInput").ap()
    oe = nc.dram_tensor("oe", [P, xfree(cap)], bf, kind="ExternalOutput").ap()
    oz = nc.dram_tensor("oz", [P, xfree(nsh)], bf, kind="ExternalOutput").ap()

    with tile.TileContext(nc) as tc, ExitStack() as ctx:
        wpool = ctx.enter_context(tc.tile_pool(name="wpool", bufs=1))
        xpool = ctx.enter_context(tc.tile_pool(name="xpool", bufs=4))
        hpool = ctx.enter_context(tc.tile_pool(name="hpool", bufs=3))
        sgpool = ctx.enter_context(tc.tile_pool(name="sgpool", bufs=3))
        opool = ctx.enter_context(tc.tile_pool(name="opool", bufs=4))
        # PSUM banks: p1 x1, p3 x1, p2 x6 (8 total)
        p1pool = ctx.enter_context(tc.tile_pool(name="p1pool", bufs=1, space="PSUM"))
        p3pool = ctx.enter_context(tc.tile_pool(name="p3pool", bufs=1, space="PSUM"))
        p2pool = ctx.enter_context(tc.tile_pool(name="p2pool", bufs=6, space="PSUM"))

        # ---- HAM warmup: dummy matmuls to trip the PE clock to 2.4 GHz ----
        wz = wpool.tile([P, 64], bf, tag="wz", name="wz")
        nc.vector.memset(wz, 0.0)
        pwarm = p2pool.tile([P, NCHUNK], f32, tag="p2", name="pwarm")
        for i in range(WARMUP_MM):
            nc.tensor.matmul(
                pwarm[:64, :64],
                wz[:, :64],
                wz[:, :64],
                start=(i == 0),
                stop=(i == WARMUP_MM - 1),
            )

        # ---- weight tiles (persistent, m-major) ----
        sw1s = wpool.tile([P, KI, KD, P], bf, tag="sw1s", name="sw1s")
        sw3s = wpool.tile([P, KI, KD, P], bf, tag="sw3s", name="sw3s")
        sw2s = wpool.tile([P, KI, KD, P], bf, tag="sw2s", name="sw2s")
        w1s = wpool.tile([P, KI, KD, P], bf, tag="w1s", name="w1s")
        w3s = wpool.tile([P, KI, KD, P], bf, tag="w3s", name="w3s")
        w2s = wpool.tile([P, KI, KD, P], bf, tag="w2s", name="w2s")

        # ---- unified chunk schedule over both phases ----
        sched = []  # (xT, outT, a1, a3, a2, xoff, n)
        xoff = 0
        for n in _chunks_of(nsh):
            sched.append((xs, oz, sw1s, sw3s, sw2s, xoff, n))
            xoff += KD * n
        xoff = 0
        for n in _chunks_of(cap):
            sched.append((xg, oe, w1s, w3s, w2s, xoff, n))
            xoff += KD * n
        nchunks = len(sched)

        # ---- DMA issue, need-ordered ----
        # x chunk 0 on the sync queue (2 halves); everything else on the
        # gpsimd queue in consumption order: shared-weight m-blocks first,
        # then sw2, x1, x2, routed weights, then the x tail.  All queues
        # feed the same 16 DMA engines; per-queue FIFO order is what
        # controls which bytes land first.
        xtiles = [None] * nchunks

        def issue_x(c, queue, split=False):
            xT, _, _, _, _, xoff, n = sched[c]
            xt = xpool.tile([P, KD, NCHUNK], bf, tag="xt", name="xt")
            src = xT[:, xoff : xoff + KD * n].rearrange("p (kt n) -> p kt n", kt=KD)
            if split:
                hh = KD // 2
                queue.dma_start(out=xt[:, :hh, :n], in_=src[:, :hh, :])
                queue.dma_start(out=xt[:, hh:, :n], in_=src[:, hh:, :])
            else:
                queue.dma_start(out=xt[:, :, :n], in_=src)
            xtiles[c] = xt

        issue_x(0, nc.sync, split=True)

        sw1r = sw1t.rearrange("p (mb k mi) -> p mb k mi", mb=KI, k=KD)
        sw3r = sw3t.rearrange("p (mb k mi) -> p mb k mi", mb=KI, k=KD)
        sw2r = sw2t.rearrange("p (mb k mi) -> p mb k mi", mb=KI, k=KD)
        # shared-weight pieces in need order: sw1 m-block m feeds step (0,m),
        # sw3 right after, sw2 k2-block feeds the w2 batch one step later
        order = [
            (sw1s, sw1r, 0), (sw3s, sw3r, 0),
            (sw1s, sw1r, 1), (sw3s, sw3r, 1), (sw2s, sw2r, 0),
            (sw1s, sw1r, 2), (sw3s, sw3r, 2), (sw2s, sw2r, 1),
            (sw1s, sw1r, 3), (sw3s, sw3r, 3), (sw2s, sw2r, 2),
        ]
        for t, src, mb in order:
            nc.gpsimd.dma_start(out=t[:, mb], in_=src[:, mb])
        issue_x(1, nc.gpsimd)
        nc.gpsimd.dma_start(out=sw2s[:, 3], in_=sw2r[:, 3])
        issue_x(2, nc.gpsimd)
        for t, src in ((w1s, w1t), (w3s, w3t), (w2s, w2t)):
            nc.gpsimd.dma_start(
                out=t[:, :, :, :],
                in_=src.rearrange("p (mb k mi) -> p mb k mi", mb=KI, k=KD),
            )

        Silu = mybir.ActivationFunctionType.Silu
        Copy = mybir.ActivationFunctionType.Copy

        htiles = [None] * nchunks
        p2ts = [None] * nchunks

        def emit_batch(c, k2):
            """k2-th accumulation slice into all KD p2 banks of chunk c."""
            _, outT, _, _, a2, xoff, n = sched[c]
            if p2ts[c] is None:
                p2ts[c] = [
                    p2pool.tile([P, NCHUNK], f32, tag="p2", name="p2")
                    for _ in range(KD)
                ]
            h = htiles[c]
            last = k2 == KI - 1
            ot = None
            if last:
                ot = opool.tile([P, KD, NCHUNK], bf, tag="ot", name="ot")
            for m2 in range(KD):
                nc.tensor.matmul(
                    p2ts[c][m2][:, :n],
                    a2[:, k2, m2, :],
                    h[:, k2, :n],
                    start=(k2 == 0),
                    stop=last,
                )
                if last:
                    # copy emitted right behind its own stop-matmul so its
                    # semaphore target is per-bank precise
                    if m2 % 2 == 0:
                        nc.scalar.activation(
                            ot[:, m2, :n], p2ts[c][m2][:, :n], Copy
                        )
                    else:
                        nc.vector.tensor_copy(ot[:, m2, :n], p2ts[c][m2][:, :n])
            if last:
                dst = outT[:, xoff : xoff + KD * n].rearrange(
                    "p (kt n) -> p kt n", kt=KD
                )
                nc.sync.dma_start(out=dst, in_=ot[:, :, :n])
                htiles[c] = None
                p2ts[c] = None
                # prefetch issue sits behind this out-DMA on the sync queue,
                # so its transfer can't steal head bandwidth
                if c + 3 < nchunks:
                    issue_x(c + 3, nc.sync)

        prev = None  # (c, m) one h-step behind
        for c in range(nchunks):
            _, _, a1, a3, _, _, n = sched[c]
            xt = xtiles[c]
            htiles[c] = hpool.tile([P, KI, NCHUNK], bf, tag="h", name="h")
            for m in range(KI):
                p1 = p1pool.tile([P, NCHUNK], f32, tag="p1", name="p1")
                for k in range(KD):
                    nc.tensor.matmul(
                        p1[:, :n],
                        a1[:, m, k, :],
                        xt[:, k, :n],
                        start=(k == 0),
                        stop=(k == KD - 1),
                    )
                sg = sgpool.tile([P, NCHUNK], bf, tag="sg", name="sg")
                nc.scalar.activation(sg[:, :n], p1[:, :n], Silu)
                p3 = p3pool.tile([P, NCHUNK], f32, tag="p3", name="p3")
                for k in range(KD):
                    nc.tensor.matmul(
                        p3[:, :n],
                        a3[:, m, k, :],
                        xt[:, k, :n],
                        start=(k == 0),
                        stop=(k == KD - 1),
                    )
                nc.vector.tensor_mul(
                    htiles[c][:, m, :n], sg[:, :n], p3[:, :n]
                )
                if prev is not None:
                    emit_batch(*prev)
                prev = (c, m)
            xtiles[c] = None
        emit_batch(*prev)

    nc.compile()
    return nc


# ------------------------------------------------------------------- driver
def kernel(x, gate_weight, gate_bias, w1, w2, w3, sw1, sw2, sw3):
    from concourse.bass_utils import run_bass_kernel_spmd

    B, S, D = x.shape
    x2 = np.ascontiguousarray(x.reshape(-1, D))
    T = x2.shape[0]
    nsh = T // N_CORES

    w, idx = _host_gate(x2, gate_weight, gate_bias)

    rows_per_e = [np.nonzero((idx == e).any(axis=1))[0] for e in range(E)]
    cap = max(len(r) for r in rows_per_e)
    cap = ((cap + P - 1) // P) * P

    nc = _build_nc(cap, nsh)

    x2T = np.ascontiguousarray(x2.T)  # [D, T]
    in_maps = []
    for e in range(E):
        rows = rows_per_e[e]
        xgT = np.zeros((DIM, cap), np.float32)
        xgT[:, : len(rows)] = x2T[:, rows]
        in_maps.append(
            {
                "xg": _swizzle_x(xgT),
                "xs": _swizzle_x(x2T[:, e * nsh : (e + 1) * nsh]),
                "w1t": _swizzle_w13(w1[e].T),
                "w3t": _swizzle_w13(w3[e].T),
                "w2t": _swizzle_w2(w2[e].T),
                "sw1t": _swizzle_w13(sw1.T),
                "sw3t": _swizzle_w13(sw3.T),
                "sw2t": _swizzle_w2(sw2.T),
            }
        )

    r = run_bass_kernel_spmd(nc, in_maps, list(range(N_CORES)))
    globals()["LAST_RESULTS"] = r
    res = r.results

    y = np.zeros((T, D), np.float32)
    for e in range(E):
        rows = rows_per_e[e]
        cnt = len(rows)
        Oe = _unswizzle_out(res[e]["oe"], cap)[:cnt]  # [cnt, D]
        we = np.where(idx[rows, 0] == e, w[rows, 0], w[rows, 1]).astype(np.float32)
        y[rows] += we[:, None] * Oe
    z = np.concatenate(
        [_unswizzle_out(res[c]["oz"], nsh) for c in range(N_CORES)], axis=0
    )
    return (y + z).reshape(B, S, D)
